# revision 17
# baseline (speedup 1.0000x reference)
"""MoE layer (E=8, top-2, D=512, H=2048) on 8 Trainium2 NeuronCores.

Expert-parallel sharding: core e holds expert e's weights and computes
its expert's FFN over the tokens routed to it (bf16 matmuls, fp32
accumulation), scaling rows by the combine weight. The gate is
replicated data-parallel: core e also computes softmax gate weights for
its 512-token slice (the gate_weights output). The host computes the
same gate in fp32 to plan the dispatch (which tokens go to which core,
i.e. the sharding), gathers each expert's tokens (capacity-padded), and
scatter-adds the per-expert outputs back into the full [N, D] output
(the unshard step).
"""

import numpy as np

import concourse.bacc as bacc
import concourse.mybir as mybir
import concourse.tile as tile
from concourse.bass_utils import run_bass_kernel_spmd

E, K, D, H = 8, 2, 512, 2048
B, S = 2, 2048
N = B * S
NS = N // 8     # tokens per core for the gate shard
DC = D // 128   # 4 contraction chunks for layer 1
HC = H // 128   # 16 contraction chunks for layer 2

_BUILT = {}     # (C, has_b2) -> finalized Bacc program
LAST_RESULTS = None  # BassKernelResults of the most recent run (for profiling)
LAST_IN_MAPS = None  # per-core input maps of the most recent run (for benching)
LAST_KEY = None      # (C, has_b2) of the most recent run


def _token_tiles(C):
    """Split capacity C (multiple of 64) into layer-1 tiles (<=512)."""
    out, off = [], 0
    while off < C:
        sz = min(512, C - off)
        out.append((off, sz))
        off += sz
    return out


def _build(C, has_b1, has_b2, reps=1):
    f32 = mybir.dt.float32
    bf16 = mybir.dt.bfloat16
    relu = mybir.ActivationFunctionType.Relu
    copy_f = mybir.ActivationFunctionType.Copy
    exp_f = mybir.ActivationFunctionType.Exp
    add_op = mybir.AluOpType.add
    max_op = mybir.AluOpType.max
    nt2 = -(-C // 128)
    ttiles = _token_tiles(C)

    nc = bacc.Bacc(target_bir_lowering=False, debug=False)
    xeT_d = nc.declare_dram_parameter("xeT", [D, C], bf16, isOutput=False)
    w1_d = nc.declare_dram_parameter("w1", [D, H], bf16, isOutput=False)
    w2_d = nc.declare_dram_parameter("w2", [H, D], bf16, isOutput=False)
    b1_d = nc.declare_dram_parameter("b1", [128, HC], f32, isOutput=False)
    b2_d = nc.declare_dram_parameter("b2", [1, D], bf16, isOutput=False)
    ge_d = nc.declare_dram_parameter("ge", [128, nt2], f32, isOutput=False)
    xsT_d = nc.declare_dram_parameter("xsT", [D, NS], bf16, isOutput=False)
    wg_d = nc.declare_dram_parameter("wg", [128, DC * E], bf16, isOutput=False)
    bg_d = nc.declare_dram_parameter("bg", [E, 1], f32, isOutput=False)
    y_d = nc.declare_dram_parameter("y", [C, D], f32, isOutput=True)
    gw_d = nc.declare_dram_parameter("gw", [E, NS], f32, isOutput=True)

    with tile.TileContext(nc) as tc:
        with (
            tc.tile_pool(name="wpool", bufs=1 if reps == 1 else 2) as wpool,
            tc.tile_pool(name="hpool", bufs=3) as hpool,
            tc.tile_pool(name="ypool", bufs=3) as ypool,
            tc.tile_pool(name="ps1", bufs=4, space="PSUM") as ps1,
            tc.tile_pool(name="ps2", bufs=3, space="PSUM") as ps2,
            tc.tile_pool(name="psg", bufs=1, space="PSUM") as psg,
        ):
          for _rep in range(reps):
            # ---- gate shard loads ----
            xsT_sb = wpool.tile([128, DC, NS], bf16, tag="xsT")
            for c in range(DC):
                nc.sync.dma_start(
                    out=xsT_sb[:, c, :], in_=xsT_d[c * 128:(c + 1) * 128, :]
                )
            wg_sb = wpool.tile([128, DC, E], bf16, tag="wg")
            nc.sync.dma_start(out=wg_sb[:, :, :], in_=wg_d[:, :])
            bg_sb = wpool.tile([E, 1], f32, tag="bg")
            nc.sync.dma_start(out=bg_sb[:, :], in_=bg_d[:, :])

            # ---- FFN loads, in first-use order ----
            # xeT: per (contraction chunk, token tile); token tile 0 first
            xeT_sb = [[None] * len(ttiles) for _ in range(DC)]
            for ti, (off, tsz) in enumerate(ttiles):
                for c in range(DC):
                    t = wpool.tile([128, tsz], bf16, tag=f"xeT_{c}_{ti}")
                    nc.sync.dma_start(
                        out=t[:, :], in_=xeT_d[c * 128:(c + 1) * 128, off:off + tsz]
                    )
                    xeT_sb[c][ti] = t
            # w1: quarters of H (so hc 0-3 can start after the first quarter)
            NQ = 4
            QW = H // NQ
            w1_sb = [[None] * NQ for _ in range(DC)]
            for q in range(NQ):
                for c in range(DC):
                    t = wpool.tile([128, QW], bf16, tag=f"w1_{c}_{q}")
                    nc.sync.dma_start(
                        out=t[:, :],
                        in_=w1_d[c * 128:(c + 1) * 128, q * QW:(q + 1) * QW],
                    )
                    w1_sb[c][q] = t
            w2_sb = []
            for c in range(HC):
                t = wpool.tile([128, D], bf16, tag=f"w2_{c}")
                nc.sync.dma_start(out=t[:, :], in_=w2_d[c * 128:(c + 1) * 128, :])
                w2_sb.append(t)
            if has_b1:
                b1_sb = wpool.tile([128, HC], f32, tag="b1")
                nc.sync.dma_start(out=b1_sb[:, :], in_=b1_d[:, :])
            ge_sb = wpool.tile([128, nt2], f32, tag="ge")
            nc.sync.dma_start(out=ge_sb[:, :], in_=ge_d[:, :])
            if has_b2:
                b2_sb = wpool.tile([1, D], bf16, tag="b2")
                nc.sync.dma_start(out=b2_sb[:, :], in_=b2_d[:, :])
                ones_sb = wpool.tile([1, 128], bf16, tag="ones")
                nc.vector.memset(ones_sb[:, :], 1.0)

            # ---- gate: gw[e, t] = softmax(x_slice @ Wg + bg)[e] ----
            pg1 = psg.tile([E, NS], f32, tag="pg")
            for c in range(DC):
                nc.tensor.matmul(
                    pg1[:, :], wg_sb[:, c, :], xsT_sb[:, c, :],
                    start=(c == 0), stop=(c == DC - 1),
                )
            e_sb = wpool.tile([E, NS], f32, tag="e_sb")
            nc.scalar.activation(e_sb[:, :], pg1[:, :], exp_f, bias=bg_sb[:, :])
            ones81 = wpool.tile([E, 1], f32, tag="ones81")
            nc.vector.memset(ones81[:, :], 1.0)
            pg2 = psg.tile([1, NS], f32, tag="pg")
            nc.tensor.matmul(pg2[:, :], ones81[:, :], e_sb[:, :])
            recip_sb = wpool.tile([1, NS], f32, tag="recip")
            nc.vector.reciprocal(recip_sb[:, :], pg2[:, :])
            ones18 = wpool.tile([1, E], f32, tag="ones18")
            nc.vector.memset(ones18[:, :], 1.0)
            pg3 = psg.tile([E, NS], f32, tag="pg")
            nc.tensor.matmul(pg3[:, :], ones18[:, :], recip_sb[:, :])
            gw_sb = wpool.tile([E, NS], f32, tag="gw")
            nc.vector.tensor_mul(gw_sb[:, :], e_sb[:, :], pg3[:, :])
            nc.sync.dma_start(out=gw_d[:, :], in_=gw_sb[:, :])

            # ---- expert FFN ----
            for ti, (off, tsz) in enumerate(ttiles):
                # layer 1: hT[h, tok] = relu(W1[:,h].T @ xT[:, tok] + b1[h])
                hT = hpool.tile([128, HC, tsz], bf16, tag="hT")
                for hc in range(HC):
                    p1 = ps1.tile([128, tsz], f32, tag="p1")
                    for c in range(DC):
                        nc.tensor.matmul(
                            p1[:, :],
                            w1_sb[c][hc // (HC // NQ)][
                                :, (hc % (HC // NQ)) * 128:
                                   (hc % (HC // NQ)) * 128 + 128],
                            xeT_sb[c][ti][:, :],
                            start=(c == 0),
                            stop=(c == DC - 1),
                        )
                    if has_b1:
                        nc.scalar.activation(
                            hT[:, hc, :], p1[:, :], relu,
                            bias=b1_sb[:, hc:hc + 1], scale=1.0,
                        )
                    elif hc % 2 == 0:
                        nc.scalar.activation(hT[:, hc, :], p1[:, :], relu)
                    else:
                        nc.vector.tensor_scalar_max(hT[:, hc, :], p1[:, :], 0.0)
                # layer 2: y[tok, :] = ge[tok] * (hT[:, tok].T @ W2 + b2)
                for s in range(0, tsz, 128):
                    m = min(128, tsz - s)
                    tok0 = off + s
                    p2 = ps2.tile([m, D], f32, tag="p2")
                    if has_b2:
                        nc.tensor.matmul(
                            p2[:, :], ones_sb[:, :m], b2_sb[:, :],
                            start=True, stop=False,
                        )
                    for hc in range(HC):
                        nc.tensor.matmul(
                            p2[:, :],
                            hT[:, hc, s:s + m],
                            w2_sb[hc][:, :],
                            start=(hc == 0 and not has_b2),
                            stop=(hc == HC - 1),
                        )
                    y_sb = ypool.tile([m, D], f32, tag="y")
                    ge_ap = ge_sb[tok0 % 128:tok0 % 128 + m,
                                  tok0 // 128:tok0 // 128 + 1]
                    if (s // 128) % 2 == 0:
                        nc.vector.tensor_scalar_mul(y_sb[:, :], p2[:, :], ge_ap)
                    else:
                        nc.scalar.activation(
                            y_sb[:, :], p2[:, :], copy_f, scale=ge_ap
                        )
                    nc.sync.dma_start(
                        out=y_d[tok0:tok0 + m, :], in_=y_sb[:, :]
                    )

    nc.finalize()
    return nc


def _bf16(a):
    import ml_dtypes
    return np.ascontiguousarray(np.asarray(a, dtype=ml_dtypes.bfloat16))


def kernel(x, Wg, bg, W1, b1, W2, b2):
    global LAST_RESULTS
    x = np.asarray(x, np.float32)
    Wg = np.asarray(Wg, np.float32)
    bg = np.asarray(bg, np.float32)
    W1 = np.asarray(W1, np.float32)
    b1 = np.asarray(b1, np.float32)
    W2 = np.asarray(W2, np.float32)
    b2 = np.asarray(b2, np.float32)

    xf = x.reshape(N, D)
    # exact fp32 gate for dispatch planning (the sharding decision)
    logits = xf @ Wg + bg
    m = logits.max(axis=-1, keepdims=True)
    ex = np.exp(logits - m)
    gate_f32 = ex / ex.sum(axis=-1, keepdims=True)
    top2 = np.argpartition(-gate_f32, K - 1, axis=-1)[:, :K]

    tok_idx, gates = [], []
    for e in range(E):
        sel = np.nonzero((top2 == e).any(axis=-1))[0]
        tok_idx.append(sel)
        gates.append(gate_f32[sel, e])
    counts = np.array([len(s) for s in tok_idx])
    C = int(-(-counts.max() // 64) * 64)
    has_b1 = bool(np.any(b1))
    has_b2 = bool(np.any(b2))

    key = (C, has_b1, has_b2)
    if key not in _BUILT:
        _BUILT[key] = _build(C, has_b1, has_b2)
    nc = _BUILT[key]

    nt2 = -(-C // 128)
    wg_staged = _bf16(
        Wg.reshape(DC, 128, E).transpose(1, 0, 2).reshape(128, DC * E)
    )
    in_maps = []
    for e in range(E):
        ne = counts[e]
        xeT = np.zeros((D, C), np.float32)
        xeT[:, :ne] = xf[tok_idx[e]].T
        ge = np.zeros((nt2 * 128,), np.float32)
        ge[:ne] = gates[e]
        in_maps.append({
            "xeT": _bf16(xeT),
            "w1": _bf16(W1[e]),
            "w2": _bf16(W2[e]),
            "b1": np.ascontiguousarray(
                b1[e].reshape(HC, 128).T.astype(np.float32)),
            "b2": _bf16(b2[e].reshape(1, D)),
            "ge": np.ascontiguousarray(
                ge.reshape(nt2, 128).T.astype(np.float32)),
            "xsT": _bf16(xf[e * NS:(e + 1) * NS].T),
            "wg": wg_staged,
            "bg": np.ascontiguousarray(bg.reshape(E, 1).astype(np.float32)),
        })

    global LAST_IN_MAPS, LAST_KEY
    LAST_IN_MAPS = in_maps
    LAST_KEY = key
    LAST_RESULTS = run_bass_kernel_spmd(nc, in_maps, list(range(E)))

    out = np.zeros((N, D), np.float32)
    gate_weights = np.zeros((N, E), np.float32)
    for e in range(E):
        ne = counts[e]
        y = np.asarray(LAST_RESULTS.results[e]["y"], np.float32)
        out[tok_idx[e]] += y[:ne]
        gate_weights[e * NS:(e + 1) * NS] = np.asarray(
            LAST_RESULTS.results[e]["gw"], np.float32).T
    return out.reshape(B, S, D), gate_weights


# revision 18
# speedup vs baseline: 1.3550x; 1.3550x over previous
"""MoE layer (E=8, top-2, D=512, H=2048) on 8 Trainium2 NeuronCores.

Expert-parallel sharding: core e holds expert e's weights and computes
its expert's FFN over the tokens routed to it (bf16 matmuls, fp32
accumulation), scaling rows by the combine weight. The gate is
replicated data-parallel: core e also computes softmax gate weights for
its 512-token slice (the gate_weights output). The host computes the
same gate in fp32 to plan the dispatch (which tokens go to which core,
i.e. the sharding), gathers each expert's tokens (capacity-padded), and
scatter-adds the per-expert outputs back into the full [N, D] output
(the unshard step).
"""

import numpy as np

import concourse.bacc as bacc
import concourse.mybir as mybir
import concourse.tile as tile
from concourse.bass_utils import run_bass_kernel_spmd

E, K, D, H = 8, 2, 512, 2048
B, S = 2, 2048
N = B * S
NS = N // 8     # tokens per core for the gate shard
DC = D // 128   # 4 contraction chunks for layer 1
HC = H // 128   # 16 contraction chunks for layer 2

_BUILT = {}     # (C, has_b1, has_b2) -> finalized Bacc program
LAST_RESULTS = None  # BassKernelResults of the most recent run (for profiling)
LAST_IN_MAPS = None  # per-core input maps of the most recent run (for benching)
LAST_KEY = None      # (C, has_b1, has_b2) of the most recent run


def _token_tiles(C):
    """Split capacity C (multiple of 64) into layer-1 tiles (<=512)."""
    out, off = [], 0
    while off < C:
        sz = min(512, C - off)
        out.append((off, sz))
        off += sz
    return out


def _build(C, has_b1, has_b2, reps=1):
    f32 = mybir.dt.float32
    bf16 = mybir.dt.bfloat16
    relu = mybir.ActivationFunctionType.Relu
    copy_f = mybir.ActivationFunctionType.Copy
    exp_f = mybir.ActivationFunctionType.Exp
    nt2 = -(-C // 128)
    ttiles = _token_tiles(C)

    nc = bacc.Bacc(target_bir_lowering=False, debug=False)
    xeT_d = nc.declare_dram_parameter("xeT", [D, C], bf16, isOutput=False)
    w1_d = nc.declare_dram_parameter("w1", [D, H], bf16, isOutput=False)
    w2_d = nc.declare_dram_parameter("w2", [H, D], bf16, isOutput=False)
    b1_d = nc.declare_dram_parameter("b1", [128, HC], f32, isOutput=False)
    b2_d = nc.declare_dram_parameter("b2", [1, D], bf16, isOutput=False)
    ge_d = nc.declare_dram_parameter("ge", [128, nt2], f32, isOutput=False)
    xsT_d = nc.declare_dram_parameter("xsT", [D, NS], bf16, isOutput=False)
    wg_d = nc.declare_dram_parameter("wg", [128, DC * E], bf16, isOutput=False)
    bg_d = nc.declare_dram_parameter("bg", [E, 1], f32, isOutput=False)
    y_d = nc.declare_dram_parameter("y", [C, D], f32, isOutput=True)
    gw_d = nc.declare_dram_parameter("gw", [E, NS], f32, isOutput=True)

    with tile.TileContext(nc) as tc:
        with (
            tc.tile_pool(name="wpool", bufs=1 if reps == 1 else 2) as wpool,
            tc.tile_pool(name="hpool", bufs=3) as hpool,
            tc.tile_pool(name="ypool", bufs=3) as ypool,
            tc.tile_pool(name="ps1", bufs=4, space="PSUM") as ps1,
            tc.tile_pool(name="ps2", bufs=3, space="PSUM") as ps2,
            tc.tile_pool(name="psg", bufs=1, space="PSUM") as psg,
        ):
          for _rep in range(reps):
            # ---- gate shard loads ----
            xsT_sb = wpool.tile([128, DC, NS], bf16, tag="xsT")
            for c in range(DC):
                nc.sync.dma_start(
                    out=xsT_sb[:, c, :], in_=xsT_d[c * 128:(c + 1) * 128, :]
                )
            wg_sb = wpool.tile([128, DC, E], bf16, tag="wg")
            nc.sync.dma_start(out=wg_sb[:, :, :], in_=wg_d[:, :])
            bg_sb = wpool.tile([E, 1], f32, tag="bg")
            nc.sync.dma_start(out=bg_sb[:, :], in_=bg_d[:, :])

            # ---- FFN loads, in first-use order ----
            # xeT: per (contraction chunk, token tile); token tile 0 first
            xeT_sb = [[None] * len(ttiles) for _ in range(DC)]
            for ti, (off, tsz) in enumerate(ttiles):
                for c in range(DC):
                    t = wpool.tile([128, tsz], bf16, tag=f"xeT_{c}_{ti}")
                    nc.sync.dma_start(
                        out=t[:, :], in_=xeT_d[c * 128:(c + 1) * 128, off:off + tsz]
                    )
                    xeT_sb[c][ti] = t
            # w1: quarters of H (so hc 0-3 can start after the first quarter)
            NQ = 4
            QW = H // NQ
            w1_sb = [[None] * NQ for _ in range(DC)]
            for q in range(NQ):
                for c in range(DC):
                    t = wpool.tile([128, QW], bf16, tag=f"w1_{c}_{q}")
                    nc.sync.dma_start(
                        out=t[:, :],
                        in_=w1_d[c * 128:(c + 1) * 128, q * QW:(q + 1) * QW],
                    )
                    w1_sb[c][q] = t
            w2_sb = []
            for c in range(HC):
                t = wpool.tile([128, D], bf16, tag=f"w2_{c}")
                nc.sync.dma_start(out=t[:, :], in_=w2_d[c * 128:(c + 1) * 128, :])
                w2_sb.append(t)
            if has_b1:
                b1_sb = wpool.tile([128, HC], f32, tag="b1")
                nc.sync.dma_start(out=b1_sb[:, :], in_=b1_d[:, :])
            ge_sb = wpool.tile([128, nt2], f32, tag="ge")
            nc.sync.dma_start(out=ge_sb[:, :], in_=ge_d[:, :])
            if has_b2:
                b2_sb = wpool.tile([1, D], bf16, tag="b2")
                nc.sync.dma_start(out=b2_sb[:, :], in_=b2_d[:, :])
                ones_sb = wpool.tile([1, 128], bf16, tag="ones")
                nc.vector.memset(ones_sb[:, :], 1.0)

            # ---- gate: gw[e, t] = softmax(x_slice @ Wg + bg)[e] ----
            pg1 = psg.tile([E, NS], f32, tag="pg")
            for c in range(DC):
                nc.tensor.matmul(
                    pg1[:, :], wg_sb[:, c, :], xsT_sb[:, c, :],
                    start=(c == 0), stop=(c == DC - 1),
                )
            e_sb = wpool.tile([E, NS], f32, tag="e_sb")
            nc.scalar.activation(e_sb[:, :], pg1[:, :], exp_f, bias=bg_sb[:, :])
            ones81 = wpool.tile([E, 1], f32, tag="ones81")
            nc.vector.memset(ones81[:, :], 1.0)
            pg2 = psg.tile([1, NS], f32, tag="pg")
            nc.tensor.matmul(pg2[:, :], ones81[:, :], e_sb[:, :])
            recip_sb = wpool.tile([1, NS], f32, tag="recip")
            nc.vector.reciprocal(recip_sb[:, :], pg2[:, :])
            ones18 = wpool.tile([1, E], f32, tag="ones18")
            nc.vector.memset(ones18[:, :], 1.0)
            pg3 = psg.tile([E, NS], f32, tag="pg")
            nc.tensor.matmul(pg3[:, :], ones18[:, :], recip_sb[:, :])
            gw_sb = wpool.tile([E, NS], f32, tag="gw")
            nc.vector.tensor_mul(gw_sb[:, :], e_sb[:, :], pg3[:, :])
            nc.sync.dma_start(out=gw_d[:, :], in_=gw_sb[:, :])

            # ---- expert FFN ----
            for ti, (off, tsz) in enumerate(ttiles):
                # layer 1: hT[h, tok] = relu(W1[:,h].T @ xT[:, tok] + b1[h])
                hT = hpool.tile([128, HC, tsz], bf16, tag="hT")
                for hc in range(HC):
                    p1 = ps1.tile([128, tsz], f32, tag="p1")
                    for c in range(DC):
                        nc.tensor.matmul(
                            p1[:, :],
                            w1_sb[c][hc // (HC // NQ)][
                                :, (hc % (HC // NQ)) * 128:
                                   (hc % (HC // NQ)) * 128 + 128],
                            xeT_sb[c][ti][:, :],
                            start=(c == 0),
                            stop=(c == DC - 1),
                        )
                    if has_b1:
                        nc.scalar.activation(
                            hT[:, hc, :], p1[:, :], relu,
                            bias=b1_sb[:, hc:hc + 1], scale=1.0,
                        )
                    elif hc % 2 == 0:
                        nc.scalar.activation(hT[:, hc, :], p1[:, :], relu)
                    else:
                        nc.vector.tensor_scalar_max(hT[:, hc, :], p1[:, :], 0.0)
                # layer 2: y[tok, :] = ge[tok] * (hT[:, tok].T @ W2 + b2)
                for s in range(0, tsz, 128):
                    m = min(128, tsz - s)
                    tok0 = off + s
                    p2 = ps2.tile([m, D], f32, tag="p2")
                    if has_b2:
                        nc.tensor.matmul(
                            p2[:, :], ones_sb[:, :m], b2_sb[:, :],
                            start=True, stop=False,
                        )
                    for hc in range(HC):
                        nc.tensor.matmul(
                            p2[:, :],
                            hT[:, hc, s:s + m],
                            w2_sb[hc][:, :],
                            start=(hc == 0 and not has_b2),
                            stop=(hc == HC - 1),
                        )
                    y_sb = ypool.tile([m, D], f32, tag="y")
                    ge_ap = ge_sb[tok0 % 128:tok0 % 128 + m,
                                  tok0 // 128:tok0 // 128 + 1]
                    if (s // 128) % 2 == 0:
                        nc.vector.tensor_scalar_mul(y_sb[:, :], p2[:, :], ge_ap)
                    else:
                        nc.scalar.activation(
                            y_sb[:, :], p2[:, :], copy_f, scale=ge_ap
                        )
                    nc.sync.dma_start(
                        out=y_d[tok0:tok0 + m, :], in_=y_sb[:, :]
                    )

    nc.finalize()
    return nc


def _bf16(a):
    import ml_dtypes
    return np.ascontiguousarray(np.asarray(a, dtype=ml_dtypes.bfloat16))


def kernel(x, Wg, bg, W1, b1, W2, b2):
    global LAST_RESULTS
    x = np.asarray(x, np.float32)
    Wg = np.asarray(Wg, np.float32)
    bg = np.asarray(bg, np.float32)
    W1 = np.asarray(W1, np.float32)
    b1 = np.asarray(b1, np.float32)
    W2 = np.asarray(W2, np.float32)
    b2 = np.asarray(b2, np.float32)

    xf = x.reshape(N, D)
    # exact fp32 gate for dispatch planning (the sharding decision)
    logits = xf @ Wg + bg
    m = logits.max(axis=-1, keepdims=True)
    ex = np.exp(logits - m)
    gate_f32 = ex / ex.sum(axis=-1, keepdims=True)
    top2 = np.argpartition(-gate_f32, K - 1, axis=-1)[:, :K]

    tok_idx, gates = [], []
    for e in range(E):
        sel = np.nonzero((top2 == e).any(axis=-1))[0]
        tok_idx.append(sel)
        gates.append(gate_f32[sel, e])
    counts = np.array([len(s) for s in tok_idx])
    C = int(-(-counts.max() // 64) * 64)
    has_b1 = bool(np.any(b1))
    has_b2 = bool(np.any(b2))

    key = (C, has_b1, has_b2)
    if key not in _BUILT:
        _BUILT[key] = _build(C, has_b1, has_b2)
    nc = _BUILT[key]

    nt2 = -(-C // 128)
    wg_staged = _bf16(
        Wg.reshape(DC, 128, E).transpose(1, 0, 2).reshape(128, DC * E)
    )
    in_maps = []
    for e in range(E):
        ne = counts[e]
        xeT = np.zeros((D, C), np.float32)
        xeT[:, :ne] = xf[tok_idx[e]].T
        ge = np.zeros((nt2 * 128,), np.float32)
        ge[:ne] = gates[e]
        in_maps.append({
            "xeT": _bf16(xeT),
            "w1": _bf16(W1[e]),
            "w2": _bf16(W2[e]),
            "b1": np.ascontiguousarray(
                b1[e].reshape(HC, 128).T.astype(np.float32)),
            "b2": _bf16(b2[e].reshape(1, D)),
            "ge": np.ascontiguousarray(
                ge.reshape(nt2, 128).T.astype(np.float32)),
            "xsT": _bf16(xf[e * NS:(e + 1) * NS].T),
            "wg": wg_staged,
            "bg": np.ascontiguousarray(bg.reshape(E, 1).astype(np.float32)),
        })

    global LAST_IN_MAPS, LAST_KEY
    LAST_IN_MAPS = in_maps
    LAST_KEY = key
    LAST_RESULTS = run_bass_kernel_spmd(nc, in_maps, list(range(E)))

    out = np.zeros((N, D), np.float32)
    gate_weights = np.zeros((N, E), np.float32)
    for e in range(E):
        ne = counts[e]
        y = np.asarray(LAST_RESULTS.results[e]["y"], np.float32)
        out[tok_idx[e]] += y[:ne]
        gate_weights[e * NS:(e + 1) * NS] = np.asarray(
            LAST_RESULTS.results[e]["gw"], np.float32).T
    return out.reshape(B, S, D), gate_weights
